# revision 52
# baseline (speedup 1.0000x reference)
"""Trainium2 Bass kernel v13 for nn_MCLMask (bipartite Katz / MCL mask).

133.3 us HW, rel err 5.0e-3 (vs 158.4 us / 5.05e-3 staged baseline).
v13 = v9 + head-only scalar offload: sup_sq and qsq(chunks 0-3) run as
scalar Square (head scalar is idle, Square shares the exp table set), and
ALL per-chunk norm headwork + both half-norm batches are emitted pre-loop
so the DVE conveyor starts packed.

Structure (all-bf16 datapath; host only reorders/casts layouts):
  - Host packs sup [128,(kshot,j,kc,s128)] (s-tiles zero-padded 125->128)
    and qry in per-chunk contiguous blocks so every DMA line is >=3KB.
  - Query 1/||q|| norms: squares+ones-matmuls per chunk, Ln/Exp batched
    per half (avoids ACT table thrash: ln and exp live in different
    table sets); result broadcast once per half.
  - Support 10/||s|| norm lands directly in COLUMN layout (1-col
    ns2s matmuls) and is applied as the per-partition *scale* of the
    e10 exp activation - no support prescale pass, shorter head.
  - e10=exp from PSUM; e20=e10^2 written contiguous; F=e10*csr strided
    into 101-wide blocks whose ones-col yields rs as a ct column; fs
    row comes from a ones-stationary matmul and is moved into ctf row
    100 by a partition-shifting SBUF->SBUF DMA.
  - Katz solve per chunk, fully on-chip, 3 injected matmul rounds:
    PSUM preloaded with 1.0 (start=False accumulate), fs coefficient
    at partition 100 of the z vectors; z-builds on DVE, x copies on
    scalar. Pipeline: iter ci emits solveA(ci-2), headwork(ci+4),
    st+exp(ci), solveB(ci-2), ct(ci-1), post(ci), solveC(ci-2) so no
    engine head-of-line blocks another.
  - Output normalized on-chip, written [M, PP]; host transposes.
"""
import sys
import os
import numpy as np
import ml_dtypes

for _p in ("/opt/trn_rl_repo",):
    if os.path.isdir(_p) and _p not in sys.path:
        sys.path.insert(0, _p)

import concourse.bass as bass
import concourse.bacc as bacc
import concourse.mybir as mybir
import concourse.tile as tile
from concourse import masks
from concourse.bass_utils import run_bass_kernel_spmd

F32 = mybir.dt.float32
BF16 = mybir.dt.bfloat16
FP8 = mybir.dt.float8e4
DR = mybir.MatmulPerfMode.DoubleRow
AX = mybir.AxisListType
OP = mybir.AluOpType
ACTF = mybir.ActivationFunctionType
LN8 = 2.0794415416798357

B_, S_, C_, HW = 4, 25, 640, 10 * 10
NW, KS = 5, 5
Q_ = 75
PP = 38
M = 100
MS = 500
CCH = 5
STIL = 4
SP = 125
FW = 101            # F block: 100 + ones col (-> rs)
EW = 104            # E20 block: 100 + ones col (-> fs row) + pad
CHUNKS = [(0, 5), (5, 5), (10, 5), (15, 5), (20, 5), (25, 5), (30, 5), (35, 3)]
QOFF = []
_o = 0
for _p0, _np in CHUNKS:
    QOFF.append(_o)
    _o += CCH * _np * M
QTOT = _o           # 19000

_CACHED = {}


def build_nc():
    nc = bacc.Bacc("TRN2", target_bir_lowering=False, debug=False)
    d_sup = nc.declare_dram_parameter("sup", [128, KS * 2560], BF16,
                                      isOutput=False)
    d_qry = nc.declare_dram_parameter("qry", [128, QTOT], BF16, isOutput=False)
    d_out = nc.declare_dram_parameter("out", [M, PP], F32, isOutput=True)

    with tile.TileContext(nc) as tc:
        from contextlib import ExitStack
        with ExitStack() as ctx:
            ek = ctx.enter_context
            p_const = ek(tc.tile_pool(name="const", bufs=1))
            p_sup = ek(tc.tile_pool(name="sup", bufs=1))
            p_sraw = ek(tc.tile_pool(name="sraw", bufs=5))
            p_big = ek(tc.tile_pool(name="big", bufs=1))
            p_qf = ek(tc.tile_pool(name="qf", bufs=8))
            p_qsq = ek(tc.tile_pool(name="qsq", bufs=4))
            p_ring = ek(tc.tile_pool(name="ring", bufs=2))
            p_tr = ek(tc.tile_pool(name="tr", bufs=2))
            p_small = ek(tc.tile_pool(name="small", bufs=1))
            p_z = ek(tc.tile_pool(name="z", bufs=2))
            p_ps = ek(tc.tile_pool(name="ps", bufs=5, space="PSUM"))
            p_ps2 = ek(tc.tile_pool(name="ps2", bufs=2, space="PSUM"))
            p_fs = ek(tc.tile_pool(name="fs", bufs=1, space="PSUM"))

            ones128b = p_const.tile([128, 1], BF16)
            nc.vector.memset(ones128b[:], 1.0)
            ones2f8 = p_const.tile([128, 2], FP8)
            nc.vector.memset(ones2f8[:], 1.0)
            ln8_b = p_const.tile([1, 1], F32)
            nc.vector.memset(ln8_b[:], LN8)

            # ---- upfront DMAs: qf0-1 first, then sup, then the rest ----
            def dma_qf(ci):
                p0, npair = CHUNKS[ci]
                W = npair * M
                qf = p_qf.tile([128, CCH * 500], BF16, tag="qf", name=f"qf{ci}")
                nc.sync.dma_start(
                    qf[:][:, 0:CCH * W],
                    d_qry[:][:, QOFF[ci]: QOFF[ci] + CCH * W])
                return qf

            qfs = {}
            sraws = []
            for k in range(KS):
                sraw = p_sraw.tile([128, 2560], BF16, tag="supraw",
                                   name=f"sraw{k}")
                nc.sync.dma_start(
                    sraw[:], d_sup[:][:, 2560 * k: 2560 * (k + 1)])
                sraws.append(sraw)
            for ci in range(8):
                qfs[ci] = dma_qf(ci)

            # ---- persistent tiles ----
            ctf = p_big.tile([FW, PP * FW], BF16)   # rows 0:100 Chat^T, 100 fs
            zc0 = p_const.tile([FW, 1], BF16)
            nc.vector.memset(zc0[:][0:100, :], 0.0)
            nc.vector.memset(zc0[:][96:101, :], 0.5)
            nc.vector.memset(zc0[:][96:100, :], 0.0)
            kq_sb = p_big.tile([M, PP], BF16)
            cs = p_small.tile([SP, STIL * PP], F32)
            csr = p_small.tile([SP, STIL * PP], F32)
            rsr_all = p_small.tile([M, PP], F32)
            nsq_row = p_small.tile([1, 4000], F32)
            a_all = p_sup.tile([128, 4000], BF16)
            CW_E = 5 * EW
            CW_F = 5 * FW

            # ---- query norm headwork (per chunk) ----
            def emit_headwork(ci):
                p0, npair = CHUNKS[ci]
                W = npair * M
                qf = qfs[ci]
                qsq = p_qsq.tile([128, CCH * 500], BF16, tag="qsq")
                if ci < 4:
                    nc.scalar.activation(qsq[:][:, 0:CCH * W],
                                         qf[:][:, 0:CCH * W], ACTF.Square)
                else:
                    nc.vector.tensor_tensor(qsq[:][:, 0:CCH * W],
                                            qf[:][:, 0:CCH * W],
                                            qf[:][:, 0:CCH * W], op=OP.mult)
                q3 = qsq[:][:, 0:CCH * W].rearrange("p (k w) -> p k w", w=W)
                ns2q_ps = p_ps2.tile([1, 500], F32, tag="ps2", name=f"nq{ci}")
                for k in range(CCH):
                    nc.tensor.matmul(ns2q_ps[:][:, 0:W], ones128b[:],
                                     qsq[:][:, k * W:(k + 1) * W],
                                     start=(k == 0), stop=(k == CCH - 1))
                nc.scalar.activation(nsq_row[:][:, 500 * ci: 500 * ci + W],
                                     ns2q_ps[:][:, 0:W], ACTF.Copy)

            def emit_half_norms(half):
                lo = 2000 * half
                hi = lo + (2000 if half == 0 else 1800)
                ln_row = p_tr.tile([1, 2000], F32, tag="lnrow")
                nc.scalar.activation(ln_row[:][:, 0:hi - lo],
                                     nsq_row[:][:, lo:hi], ACTF.Ln)
                a_row = p_tr.tile([1, 2000], BF16, tag="arow")
                nc.scalar.activation(a_row[:][:, 0:hi - lo],
                                     ln_row[:][:, 0:hi - lo], ACTF.Exp,
                                     scale=-0.5, bias=ln8_b[:])
                for ci in range(4 * half, 4 * half + 4):
                    W = CHUNKS[ci][1] * M
                    off = 500 * ci - lo
                    nc.gpsimd.partition_broadcast(
                        a_all[:][:, 500 * ci: 500 * ci + W],
                        a_row[:][:, off: off + W])

            state = {}

            def emit_solve_a(ci):
                p0, npair = CHUNKS[ci]
                x_ps = p_ps.tile([M, 15], F32, tag="ps", name=f"x{ci}")
                state[("x", ci)] = x_ps
                nc.vector.memset(x_ps[:][:, 0:10], 1.0)
                for p in range(npair):
                    col = FW * (p0 + p)
                    nc.tensor.matmul(x_ps[:][:, p:p + 1],
                                     ctf[:][64:101, col:col + M],
                                     zc0[:][64:101, :], start=False, stop=True,
                                     skip_group_check=True)
                x0_sb = p_z.tile([M, 5], F32, tag="x0sb")
                state[("x0", ci)] = x0_sb
                nc.scalar.activation(x0_sb[:][:, 0:npair], x_ps[:][:, 0:npair],
                                     ACTF.Copy, scale=0.25)

            def emit_z1(ci):
                p0, npair = CHUNKS[ci]
                z1t = p_z.tile([FW, 5], BF16, tag="z1")
                state[("z1", ci)] = z1t
                nc.vector.memset(z1t[:][96:101, :], 0.5)
                nc.vector.tensor_tensor(
                    z1t[:][0:M, 0:npair], state[("x0", ci)][:][:, 0:npair],
                    rsr_all[:][:, p0:p0 + npair], op=OP.mult)

            def emit_solve_b(ci):
                p0, npair = CHUNKS[ci]
                x_ps = state[("x", ci)]
                z1t = state[("z1", ci)]
                for p in range(npair):
                    col = FW * (p0 + p)
                    nc.tensor.matmul(x_ps[:][:, 5 + p:6 + p],
                                     ctf[:][0:101, col:col + M],
                                     z1t[:][:, p:p + 1], start=False, stop=True,
                                     skip_group_check=True)
                x1_sb = p_z.tile([M, 5], F32, tag="x1sb")
                state[("x1", ci)] = x1_sb
                nc.scalar.activation(x1_sb[:][:, 0:npair],
                                     x_ps[:][:, 5:5 + npair],
                                     ACTF.Copy, scale=0.25)

            def emit_z2(ci):
                p0, npair = CHUNKS[ci]
                z2t = p_z.tile([FW, 5], BF16, tag="z2")
                state[("z2", ci)] = z2t
                nc.vector.memset(z2t[:][96:101, :], 0.5)
                nc.vector.tensor_tensor(
                    z2t[:][0:M, 0:npair], state[("x1", ci)][:][:, 0:npair],
                    rsr_all[:][:, p0:p0 + npair], op=OP.mult)

            def emit_solve_c(ci):
                p0, npair = CHUNKS[ci]
                x_ps = state[("x", ci)]
                z2t = state[("z2", ci)]
                for p in range(npair):
                    col = FW * (p0 + p)
                    nc.tensor.matmul(x_ps[:][:, 10 + p:11 + p],
                                     ctf[:][0:101, col:col + M],
                                     z2t[:][:, p:p + 1], start=True, stop=True)
                nc.scalar.activation(kq_sb[:][:, p0:p0 + npair],
                                     x_ps[:][:, 10:10 + npair], ACTF.Copy)

            def emit_ct(ci):
                p0, npair = CHUNKS[ci]
                Wf = npair * FW
                e20c, ftc = state[("ef", ci)]
                ct_ps = p_ps2.tile([M, 5 * FW], F32, tag="ps2", name=f"ct{ci}")
                for i in range(npair):
                    for j in range(STIL):
                        nc.tensor.matmul(
                            ct_ps[:][:, FW * i:FW * (i + 1)],
                            e20c[:][:, 500 * j + M * i: 500 * j + M * (i + 1)],
                            ftc[:][:, CW_F * j + FW * i: CW_F * j + FW * (i + 1)],
                            start=(j == 0), stop=(j == STIL - 1))
                fs_ps = p_fs.tile([1, 5 * FW], F32, tag="fs", name=f"fs{ci}")
                for j in range(STIL):
                    nc.tensor.matmul(
                        fs_ps[:][:, 0:Wf], ones128b[:][0:SP, :],
                        ftc[:][:, CW_F * j: CW_F * j + Wf],
                        start=(j == 0), stop=(j == STIL - 1))
                nc.scalar.activation(
                    ctf[:][0:M, FW * p0: FW * p0 + Wf],
                    ct_ps[:][0:M, 0:Wf], ACTF.Copy)
                fs_sb = p_z.tile([1, 5 * FW], BF16, tag="fssb")
                nc.scalar.activation(fs_sb[:][:, 0:Wf], fs_ps[:][:, 0:Wf],
                                     ACTF.Copy)
                nc.sync.dma_start(ctf[:][100:101, FW * p0: FW * p0 + Wf],
                                  fs_sb[:][:, 0:Wf])
                rs_v = ctf[:][0:M, :].rearrange(
                    "a (p r) -> a p r", r=FW)[:, p0:p0 + npair, 100:101]
                rs_f = p_z.tile([M, 5], F32, tag="rsf")
                nc.vector.tensor_copy(rs_f[:][:, 0:npair].unsqueeze(2), rs_v)
                nc.vector.reciprocal_approx_fast(
                    rsr_all[:][:, p0:p0 + npair], rs_f[:][:, 0:npair])

            def emit_post1(ci):
                p0, npair = CHUNKS[ci]
                e10c = state[("e10", ci)]
                e20c, _ = state[("ef", ci)]
                for j in range(STIL):
                    src = e10c[:][:, 500 * j: 500 * j + M * npair]
                    srcr = src.rearrange("s (p m) -> s p m", m=M)
                    eng = nc.gpsimd if j >= 2 else nc.vector
                    eng.tensor_tensor(
                        e20c[:][:, 500 * j: 500 * j + M * npair],
                        src, src, op=OP.mult)
                    nc.vector.tensor_reduce(
                        cs[:][:, PP * j + p0: PP * j + p0 + npair],
                        srcr, axis=AX.X, op=OP.add)
                cs_sl = cs[:].rearrange("s (j p) -> s j p", j=STIL)[:, :, p0:p0 + npair]
                csr_sl = csr[:].rearrange("s (j p) -> s j p", j=STIL)[:, :, p0:p0 + npair]
                nc.vector.reciprocal_approx_fast(csr_sl, cs_sl)

            def emit_post2(ci):
                p0, npair = CHUNKS[ci]
                e10c = state[("e10", ci)]
                _, ftc = state[("ef", ci)]
                for j in range(STIL):
                    src = e10c[:][:, 500 * j: 500 * j + M * npair]
                    srcr = src.rearrange("s (p m) -> s p m", m=M)
                    dst = ftc[:][:, CW_F * j: CW_F * j + FW * npair]
                    dstr = dst.rearrange("s (p r) -> s p r", r=FW)[:, :, 0:100]
                    eng = nc.vector if j < 2 else nc.gpsimd
                    eng.tensor_tensor(
                        dstr, srcr,
                        csr[:][:, PP * j + p0: PP * j + p0 + npair]
                        .unsqueeze(2).broadcast_to((SP, npair, M)),
                        op=OP.mult)

            # ---- pre-loop headwork: all chunks ----
            for ci in range(4):
                emit_headwork(ci)
            emit_half_norms(0)
            for ci in range(4, 8):
                emit_headwork(ci)
            emit_half_norms(1)

            # ---- support prep (emitted early; runs while queries land) ----
            sup_acc = p_sup.tile([128, 2560], BF16)
            s01 = p_tr.tile([128, 2560], BF16, tag="s01")
            nc.vector.tensor_tensor(s01[:], sraws[0][:], sraws[1][:], op=OP.add)
            nc.vector.tensor_tensor(s01[:], s01[:], sraws[2][:], op=OP.add)
            nc.vector.tensor_tensor(s01[:], s01[:], sraws[3][:], op=OP.add)
            nc.vector.tensor_tensor(sup_acc[:], s01[:], sraws[4][:], op=OP.add)
            sup_sq = p_sup.tile([128, 2560], BF16)
            nc.scalar.activation(sup_sq[:], sup_acc[:], ACTF.Square)
            ns2s_ps = p_ps2.tile([SP, STIL], F32, tag="ps2", name="ns2s")
            for j in range(STIL):
                for k in range(CCH):
                    nc.tensor.matmul(
                        ns2s_ps[:][:, j:j + 1],
                        sup_sq[:][:, 640 * j + 128 * k: 640 * j + 128 * k + SP],
                        ones128b[:], start=(k == 0), stop=(k == CCH - 1))
            snr_ln = p_small.tile([SP, STIL], F32)
            nc.scalar.activation(snr_ln[:], ns2s_ps[:], ACTF.Ln, scale=0.64)
            snr_col = p_small.tile([SP, STIL], F32)
            nc.scalar.activation(snr_col[:], snr_ln[:], ACTF.Exp, scale=-0.5)


            # ---- main pipeline ----
            for ci, (p0, npair) in enumerate(CHUNKS):
                W = npair * M
                qf = qfs[ci]
                if ci >= 2:
                    emit_solve_a(ci - 2)

                qsc = p_tr.tile([128, CCH * 512], BF16, tag="qsc")
                qsc3 = qsc[:].rearrange("p (k w) -> p k w", w=512)
                nc.vector.tensor_tensor(
                    qsc3[:, :, 0:W],
                    qf[:][:, 0:CCH * W].rearrange("p (k w) -> p k w", w=W),
                    a_all[:][:, 500 * ci: 500 * ci + W]
                    .unsqueeze(1).broadcast_to((128, CCH, W)),
                    op=OP.mult)
                if ci >= 2:
                    emit_z1(ci - 2)

                e10c = p_ring.tile([SP, STIL * 500], BF16, tag="e10c")
                e20c = p_ring.tile([SP, STIL * 500], BF16, tag="e20c")
                ftc = p_ring.tile([SP, STIL * CW_F], BF16, tag="ftc")
                state[("e10", ci)] = e10c
                state[("ef", ci)] = (e20c, ftc)
                nc.vector.memset(
                    ftc[:].rearrange("s (b r) -> s b r", r=FW)[:, :, 100:101], 1.0)
                for j in range(STIL):
                    st_ps = p_ps.tile([128, 500], F32, tag="ps", name=f"st{ci}_{j}")
                    for k in range(CCH):
                        nc.tensor.matmul(
                            st_ps[:][:, 0:W],
                            sup_acc[:][:, 640 * j + 128 * k: 640 * j + 128 * (k + 1)],
                            qsc[:][:, 512 * k: 512 * k + W],
                            start=(k == 0), stop=(k == CCH - 1))
                    nc.scalar.activation(
                        e10c[:][:, 500 * j: 500 * j + W], st_ps[:][0:SP, 0:W],
                        ACTF.Exp, scale=snr_col[:][:, j:j + 1])

                if ci >= 2:
                    emit_solve_b(ci - 2)
                if ci >= 1:
                    emit_ct(ci - 1)
                emit_post1(ci)
                if ci >= 2:
                    emit_z2(ci - 2)
                    emit_solve_c(ci - 2)
                emit_post2(ci)

            # ---- drain: ct(7), solve(6), solve(7) ----
            emit_solve_a(6)
            emit_ct(7)
            emit_z1(6)
            emit_solve_b(6)
            emit_solve_a(7)
            emit_z1(7)
            emit_z2(6)
            emit_solve_c(6)
            emit_solve_b(7)
            emit_z2(7)
            emit_solve_c(7)

            # ---- normalize + output ----
            ssum_ps = p_ps2.tile([1, PP], F32, tag="ps2", name="ssum")
            nc.tensor.matmul(ssum_ps[:], ones128b[:][0:M, :], kq_sb[:],
                             start=True, stop=True)
            sinv = p_small.tile([1, PP], F32)
            nc.vector.reciprocal_approx_fast(sinv[:], ssum_ps[:])
            sinv_bc = p_small.tile([M, PP], F32)
            nc.gpsimd.partition_broadcast(sinv_bc[:], sinv[:])
            out_t = p_small.tile([M, PP], F32)
            nc.vector.tensor_tensor(out_t[:], kq_sb[:], sinv_bc[:], op=OP.mult)
            nc.sync.dma_start(d_out[:], out_t[:])

    nc.compile()
    return nc


def shard_inputs(support_xf, query_xf):
    support_xf = np.asarray(support_xf, dtype=np.float32)
    query_xf = np.asarray(query_xf, dtype=np.float32)
    in_maps = []
    for core in range(8):
        b = core // 2
        half = core % 2
        qs = np.clip(np.arange(half * PP, half * PP + PP), 0, Q_ - 1)
        # [128, (kshot, j, kc, s125)] so DR k-pair subtiles are contiguous
        sup_r = (support_xf[b].reshape(NW, KS, CCH, 128, HW)
                 .transpose(3, 1, 2, 0, 4)          # [128, KS, CCH, NW, HW]
                 .reshape(128, KS, CCH, STIL, SP))  # s = (j, 125)
        sup_r = np.pad(sup_r, ((0, 0), (0, 0), (0, 0), (0, 0), (0, 3)))
        sup_r = sup_r.transpose(0, 1, 3, 2, 4)      # [128, KS, j, CCH, 128]
        sup = np.ascontiguousarray(
            sup_r.reshape(128, KS * 2560)).astype(ml_dtypes.bfloat16)
        qr = query_xf[b, qs].reshape(PP, CCH, 128, HW)
        cols = []
        for (p0, npair) in CHUNKS:
            blk = qr[p0:p0 + npair]
            cols.append(blk.transpose(2, 1, 0, 3).reshape(128, CCH * npair * HW))
        qry = np.ascontiguousarray(np.concatenate(cols, axis=1)
                                   ).astype(ml_dtypes.bfloat16)
        in_maps.append({"sup": sup, "qry": qry})
    return in_maps


def run_sharded(support_xf, query_xf, trace=False, **kw):
    if "nc" not in _CACHED:
        _CACHED["nc"] = build_nc()
    nc = _CACHED["nc"]
    in_maps = shard_inputs(support_xf, query_xf)
    res = run_bass_kernel_spmd(nc, in_maps, core_ids=list(range(8)), trace=trace, **kw)
    b, q = np.asarray(support_xf).shape[0], np.asarray(query_xf).shape[1]
    out = np.zeros((b, q, 1, 10, 10), np.float32)
    for core in range(8):
        bi = core // 2
        half = core % 2
        real = min(PP, q - half * PP)
        o = res.results[core]["out"]          # [M, PP]
        out[bi, half * PP: half * PP + real] = (
            o[:, :real].T.reshape(real, 1, 10, 10))
    return out, res


def kernel(support_xf, query_xf, n_way=5, k_shot=5):
    out, _ = run_sharded(support_xf, query_xf, trace=False)
    return out
